# revision 1
# baseline (speedup 1.0000x reference)
"""Differential attention (B=2, T=2048, D=2048, H=16, HD=128) on 8 TRN2 cores.

Sharding: core c -> (batch b = c // 4, head-group g = c % 4); each core runs
batch b with 4 heads (4g..4g+3). Out-projection partials are summed on host
(outputs stored bf16; host accumulates in f32).

Per-core schedule (single SPMD Bass program), built around keeping the PE's
in-order queue from ever head-blocking on the softmax pipeline:
  - batched DMAs: x^T / weights host-rearranged so every SBUF tile loads from
    contiguous DRAM; leading chunks are small so the first Q-projection
    matmuls start ~6us in; loads split across the SP and Activation HWDGE
    queues.
  - causal diff-attention per (head, 512-row q-superblock): scores via
    row-grouped K=64 matmul pairs into a 3-deep PSUM ring, exp on ScalarE
    with fused row-sum accumulators (one merged [128,8] stats tile per
    q-tile; single reduce + single reciprocal for both softmax sums),
    combine as one scalar_tensor_tensor (c*e2 - e1, c = lam*l1/l2).
  - the 1/l1 normalization and the combine sign ride the transpose for free:
    transposes are REGULAR matmuls streaming diag(-1/l1) built by one
    [128,128] tensor_scalar (is_transpose ignores rhs values, so it cannot
    be used for this).
  - attn @ V runs one s-chunk behind the transposes (psum-evac latency
    hidden); attention-out columns are evacuated per 128-row block as soon
    as their last AV contribution lands, so the final out-projection weaves
    into the last TAV.
  - WEAVE scheduler: score chunks are interleaved at ~chunk granularity with
    independent PE work (previous head's transpose+AV, later t-block
    projections, out-projections) via cost-tagged generators, sized so the
    PE never drains while ScalarE/DVE chew through softmax (PE ~97% busy
    mid-kernel).

All matmuls bf16 (fp8 rejected: measured rel-err 3.6e-2+ against the 2e-2
budget); accumulation fp32 in PSUM, softmax statistics fp32.
"""

from contextlib import ExitStack

import ml_dtypes
import numpy as np

B, T, D = 2, 2048, 2048
H, HD = 16, 128
HHD = HD // 2
HL = 4  # heads per core
NCORES = 8
SCALE = 1.0 / float(np.sqrt(np.float32(HHD)))

TB = 512  # t-superblock (q-block rows, AV free dim)
NTB = T // TB  # 4
DC = 128  # contraction chunk (partition dim)
NDC = D // DC  # 16
NQT = TB // 128  # q-tiles (128 rows) per superblock
CH = 512  # softmax chunk width (1 PSUM bank)
XCH = 8  # x dma chunks per t-block

_CACHE = {}


def _build():
    import concourse.mybir as mybir
    from concourse.bacc import Bacc
    from concourse.tile import TileContext

    f32 = mybir.dt.float32
    bf16 = mybir.dt.bfloat16
    Alu = mybir.AluOpType
    Act = mybir.ActivationFunctionType
    X = mybir.AxisListType.X

    nc = Bacc("TRN2", num_devices=NCORES)
    # host-rearranged inputs: xr[p, tb*8192 + d*512 + j] = x[b][tb*512+j, d*128+p]
    xr = nc.declare_dram_parameter("xr", [128, NTB * NDC * TB], bf16, isOutput=False)
    # wq/wk/wv: w_r[p, d*512 + c] = W[d*128+p, g*512 + c]
    wq = nc.declare_dram_parameter("wq", [128, NDC * HL * HD], bf16, isOutput=False)
    wk = nc.declare_dram_parameter("wk", [128, NDC * HL * HD], bf16, isOutput=False)
    wv = nc.declare_dram_parameter("wv", [128, NDC * HL * HD], bf16, isOutput=False)
    # won_r[p, h*2048 + n] = Wo[g*512 + h*128 + p, n]
    won = nc.declare_dram_parameter("won", [128, HL * D], bf16, isOutput=False)
    lam = nc.declare_dram_parameter("lam", [128, HL], f32, isOutput=False)
    msk = nc.declare_dram_parameter("msk", [128, 128], f32, isOutput=False)
    idnn = nc.declare_dram_parameter("idnn", [128, 128], bf16, isOutput=False)
    out = nc.declare_dram_parameter("out", [T, D], bf16, isOutput=True)

    with TileContext(nc) as tc, ExitStack() as top:
        # ---- persistent SBUF ----
        const = top.enter_context(tc.tile_pool(name="const", bufs=1))
        lam_sb = const.tile([128, HL], f32, tag="lam", name="lam")
        msk_sb = const.tile([128, 128], f32, tag="msk", name="msk")
        idnn_sb = const.tile([128, 128], bf16, tag="idnn", name="idnn")

        wpool = top.enter_context(tc.tile_pool(name="wpool", bufs=1))
        wq_sb = wpool.tile([128, NDC * HL * HD], bf16, tag="wq", name="wq")
        wk_sb = wpool.tile([128, NDC * HL * HD], bf16, tag="wk", name="wk")
        wv_sb = wpool.tile([128, NDC * HL * HD], bf16, tag="wv", name="wv")
        wo_sb = wpool.tile([128, HL * D], bf16, tag="wo", name="wo")

        resid = top.enter_context(tc.tile_pool(name="resid", bufs=1))
        kt_sb = [resid.tile([128, T], bf16, tag=f"kt{h}", name=f"kt{h}") for h in range(HL)]
        v_sb = [resid.tile([128, HL * HD], bf16, tag=f"v{s}", name=f"v{s}") for s in range(T // 128)]
        ot_sb = [resid.tile([128, T], bf16, tag=f"ot{h}", name=f"ot{h}") for h in range(HL)]

        # q tiles per (h, tb): alive ~1.5 superblocks -> 8-buf rotation
        qpool = top.enter_context(tc.tile_pool(name="qpool", bufs=2 * HL))
        qt_sb = {}

        xpool = top.enter_context(tc.tile_pool(name="xpool", bufs=2))
        x_sb = {}

        # ---- input DMAs: x(tb0) + wq first (PE starts on them) ----
        # x tile per tb is [128, NDC*TB] (d-major), loaded in XCH chunks
        def emit_xdma(tb):
            xt = xpool.tile([128, NDC * TB], bf16, tag="x", name=f"x{tb}")
            if tb == 0:
                widths = [128, 128, 256, 512] + [1024] * 7
            else:
                widths = [1024] * 8
            off = 0
            for cw in widths:
                nc.sync.dma_start(
                    out=xt[:, off : off + cw],
                    in_=xr[:, tb * NDC * TB + off : tb * NDC * TB + off + cw],
                )
                off += cw
            x_sb[tb] = xt

        emit_xdma(0)
        off = 0
        for cw in (512, 512, 1024, 2048, 4096):
            nc.scalar.dma_start(out=wq_sb[:, off : off + cw], in_=wq[:, off : off + cw])
            off += cw
        nc.sync.dma_start(out=lam_sb[:], in_=lam[:])
        nc.sync.dma_start(out=msk_sb[:], in_=msk[:])
        nc.sync.dma_start(out=idnn_sb[:], in_=idnn[:])
        qw = NDC * HL * HD // 4
        for k in range(4):
            nc.scalar.dma_start(out=wk_sb[:, k * qw : (k + 1) * qw], in_=wk[:, k * qw : (k + 1) * qw])
        for k in range(4):
            nc.scalar.dma_start(out=wv_sb[:, k * qw : (k + 1) * qw], in_=wv[:, k * qw : (k + 1) * qw])
        ow = HL * D // 2
        for k in range(2):
            nc.scalar.dma_start(out=wo_sb[:, k * ow : (k + 1) * ow], in_=won[:, k * ow : (k + 1) * ow])

        # ---- projection emitters (pe fillers) ----
        def emit_q(tb, h, psp):
            """Q-projection for one (t-block, head): qt[hd=128, 512 rows]."""
            xt = x_sb[tb]
            ps = psp.tile([128, TB], f32, tag="ps", name="pj")
            for d in range(NDC):
                nc.tensor.matmul(
                    ps[:],
                    lhsT=wq_sb[:, d * 512 + h * HD : d * 512 + (h + 1) * HD],
                    rhs=xt[:, d * TB : (d + 1) * TB],
                    start=(d == 0),
                    stop=(d == NDC - 1),
                )
            qt = qpool.tile([128, TB], bf16, tag="qt", name=f"qt{tb}_{h}")
            nc.vector.tensor_copy(qt[:], ps[:])
            qt_sb[(tb, h)] = qt

        def emit_k(tb, h, psp):
            xt = x_sb[tb]
            ps = psp.tile([128, TB], f32, tag="ps", name="pj")
            for d in range(NDC):
                nc.tensor.matmul(
                    ps[:],
                    lhsT=wk_sb[:, d * 512 + h * HD : d * 512 + (h + 1) * HD],
                    rhs=xt[:, d * TB : (d + 1) * TB],
                    start=(d == 0),
                    stop=(d == NDC - 1),
                )
            nc.scalar.copy(kt_sb[h][:, tb * TB : (tb + 1) * TB], ps[:])

        def emit_v(tb, tt, psp):
            """V-projection for one 128-row slab: v[s=128, HL*HD]."""
            xt = x_sb[tb]
            ps = psp.tile([128, HL * HD], f32, tag="ps", name="pj")
            for d in range(NDC):
                nc.tensor.matmul(
                    ps[:],
                    lhsT=xt[:, d * TB + tt * 128 : d * TB + (tt + 1) * 128],
                    rhs=wv_sb[:, d * 512 : (d + 1) * 512],
                    start=(d == 0),
                    stop=(d == NDC - 1),
                )
            nc.scalar.copy(v_sb[tb * NQT + tt][:], ps[:])

        # ---- prologue: Q(tb0), K(tb0) on a wide psum pool ----
        with ExitStack() as ph1:
            pps = ph1.enter_context(tc.tile_pool(name="pps", bufs=8, space="PSUM"))
            xt0 = x_sb[0]
            for w_sb, store in ((wq_sb, "q"), (wk_sb, "k")):
                pss = [pps.tile([128, TB], f32, tag="ps", name=f"p{h}", bufs=4) for h in range(HL)]
                for d in range(NDC):
                    for h in range(HL):
                        nc.tensor.matmul(
                            pss[h][:],
                            lhsT=w_sb[:, d * 512 + h * HD : d * 512 + (h + 1) * HD],
                            rhs=xt0[:, d * TB : (d + 1) * TB],
                            start=(d == 0),
                            stop=(d == NDC - 1),
                        )
                for h in range(HL):
                    if store == "q":
                        qt = qpool.tile([128, TB], bf16, tag="qt", name=f"qt0_{h}")
                        nc.vector.tensor_copy(qt[:], pss[h][:])
                        qt_sb[(0, h)] = qt
                    else:
                        nc.vector.tensor_copy(kt_sb[h][:, 0:TB], pss[h][:])

        # ---------------- attention + late projections ----------------
        # Emission = per-engine execution order (in-order queues). To keep the
        # PE from head-blocking on the softmax pipeline, score chunks are
        # WOVEN with independent PE work (previous head's transpose+AV,
        # later-t-block projections, out-projections) at ~chunk granularity.
        from collections import deque

        with ExitStack() as ph2:
            scps = ph2.enter_context(tc.tile_pool(name="scps", bufs=3, space="PSUM"))
            prjps = ph2.enter_context(tc.tile_pool(name="prjps", bufs=2, space="PSUM"))
            atps = ph2.enter_context(tc.tile_pool(name="atps", bufs=2, space="PSUM"))
            accps = ph2.enter_context(tc.tile_pool(name="accps", bufs=1, space="PSUM"))
            epool = ph2.enter_context(tc.tile_pool(name="epool", bufs=10))
            dpool = ph2.enter_context(tc.tile_pool(name="dpool", bufs=34))
            apool = ph2.enter_context(tc.tile_pool(name="apool", bufs=3))
            opool = ph2.enter_context(tc.tile_pool(name="opool", bufs=4))
            spool = ph2.enter_context(tc.tile_pool(name="spool", bufs=4))
            gpool = ph2.enter_context(tc.tile_pool(name="gpool", bufs=8))

            def score_gen(h, qsb, diffs, diags):
                """scores+exp+stats+combine for (h, qsb); yields per chunk."""
                qt_t = qt_sb[(qsb, h)]
                q1 = qt_t[0:64, :]
                q2 = qt_t[64:128, :]
                k1 = kt_sb[h][0:64, :]
                k2 = kt_sb[h][64:128, :]
                for qt in range(NQT):
                    S = qsb * TB + qt * 128 + 128
                    nch = (S + CH - 1) // CH
                    l12p = spool.tile([128, 8], f32, tag="l12p", name="l12p")
                    chunks = [None] * nch
                    # diagonal (mask-hop) chunk first: its DVE mask + exps
                    # overlap the plain chunks' matmuls/exps
                    for c in ([nch - 1] + list(range(nch - 1)) if nch > 1 else [0]):
                        w = min(CH, S - c * CH)
                        ps1 = scps.tile([128, CH], f32, tag="ps", name="ps1")
                        ps2 = scps.tile([128, CH], f32, tag="ps", name="ps2")
                        nc.tensor.matmul(
                            ps1[:, :w], lhsT=q1[:, qt * 128 : (qt + 1) * 128],
                            rhs=k1[:, c * CH : c * CH + w], start=True, stop=True,
                        )
                        nc.tensor.matmul(
                            ps2[:, :w], lhsT=q2[:, qt * 128 : (qt + 1) * 128],
                            rhs=k2[:, c * CH : c * CH + w], start=True, stop=True,
                        )
                        if c == nch - 1:
                            dw = w - 128
                            nc.vector.tensor_add(
                                ps1[:, dw : dw + 128], ps1[:, dw : dw + 128], msk_sb[:]
                            )
                            nc.vector.tensor_add(
                                ps2[:, dw : dw + 128], ps2[:, dw : dw + 128], msk_sb[:]
                            )
                        e1 = epool.tile([128, CH], bf16, tag="e", name="e1")
                        e2 = epool.tile([128, CH], bf16, tag="e", name="e2")
                        nc.scalar.activation(
                            e1[:, :w], ps1[:, :w], Act.Exp, scale=SCALE,
                            accum_out=l12p[:, c : c + 1],
                        )
                        nc.scalar.activation(
                            e2[:, :w], ps2[:, :w], Act.Exp, scale=SCALE,
                            accum_out=l12p[:, 4 + c : 5 + c],
                        )
                        chunks[c] = (e1, e2, w)
                        yield 430

                    # per-qt stats: cc = lam*l1/l2, diag = -I * (1/l1)
                    if nch > 1:
                        l12 = spool.tile([128, 2], f32, tag="l12", name="l12")
                        nc.vector.reduce_sum(
                            l12[:], l12p[:].rearrange("p (a c) -> p a c", a=2)[:, :, :nch],
                            axis=X,
                        )
                        s1, s12 = l12[:, 0:1], l12[:]
                    else:
                        s1, s12 = l12p[:, 0:1], l12p[:].rearrange("p (a c) -> p a c", a=2)[:, :, 0]
                    rl12 = spool.tile([128, 2], f32, tag="rl12", name="rl12")
                    cc = spool.tile([128, 1], f32, tag="cc", name="cc")
                    nc.vector.reciprocal(rl12[:], s12)
                    nc.vector.scalar_tensor_tensor(
                        cc[:], s1, lam_sb[:, h : h + 1], rl12[:, 1:2], Alu.mult, Alu.mult
                    )
                    r1 = rl12[:, 0:1]
                    diag = gpool.tile([128, 128], bf16, tag="dg", name="dg")
                    nc.vector.tensor_scalar(diag[:], idnn_sb[:], r1, None, Alu.mult)
                    dchunks = []
                    for e1, e2, w in chunks:
                        dn = dpool.tile([128, CH], bf16, tag="dn", name="dn")
                        nc.vector.scalar_tensor_tensor(
                            dn[:, :w], e2[:, :w], cc[:], e1[:, :w],
                            Alu.mult, Alu.subtract,
                        )
                        dchunks.append((dn, w))
                    diffs.append(dchunks)
                    diags.append(diag)

            def tav_gen(h, qsb, diffs, diags):
                """diag-matmul transposes + attn@V (AV one k behind) + ot copy."""
                nsc = (qsb + 1) * NQT
                av = accps.tile([128, TB], f32, tag="acc", name="av")

                def emit_av(pk, pj0, paTs):
                    nc.tensor.matmul(
                        av[:, pj0 * 128 :],
                        lhsT=v_sb[pk][:, h * HD : (h + 1) * HD],
                        rhs=paTs[:, pj0 * 128 :],
                        start=(pk == 0),
                        stop=(pk == nsc - 1),
                    )
                    if pk >= qsb * NQT:
                        # column block tq has no later contributors: evacuate
                        tq = pk - qsb * NQT
                        nc.vector.tensor_copy(
                            ot_sb[h][:, qsb * TB + tq * 128 : qsb * TB + (tq + 1) * 128],
                            av[:, tq * 128 : (tq + 1) * 128],
                        )

                prev = None
                for k in range(nsc):
                    j0 = 0 if k < qsb * NQT else (k - qsb * NQT)
                    aT = atps.tile([128, TB], f32, tag="aT", name="aT")
                    for j in range(j0, NQT):
                        c, off = divmod(k * 128, CH)
                        dn, _w = diffs[j][c]
                        nc.tensor.matmul(
                            aT[:, j * 128 : (j + 1) * 128],
                            lhsT=dn[:, off : off + 128],
                            rhs=diags[j][:],
                            start=True,
                            stop=True,
                        )
                    aTs = apool.tile([128, TB], bf16, tag="aTs", name="aTs")
                    if qsb <= 1:
                        nc.scalar.copy(aTs[:, j0 * 128 :], aT[:, j0 * 128 :])
                    else:
                        nc.vector.tensor_copy(aTs[:, j0 * 128 :], aT[:, j0 * 128 :])
                    if prev is not None:
                        emit_av(*prev)
                    prev = (k, j0, aTs)
                    yield 430
                emit_av(*prev)

            def q_gen(tb, h):
                xt = x_sb[tb]
                ps = prjps.tile([128, TB], f32, tag="pj", name="pj")
                for d in range(NDC):
                    nc.tensor.matmul(
                        ps[:],
                        lhsT=wq_sb[:, d * 512 + h * HD : d * 512 + (h + 1) * HD],
                        rhs=xt[:, d * TB : (d + 1) * TB],
                        start=(d == 0),
                        stop=(d == NDC - 1),
                    )
                    if d % 2 == 1:
                        yield 430
                qt = qpool.tile([128, TB], bf16, tag="qt", name=f"qt{tb}_{h}")
                nc.scalar.copy(qt[:], ps[:])
                qt_sb[(tb, h)] = qt

            def k_gen(tb, h):
                xt = x_sb[tb]
                ps = prjps.tile([128, TB], f32, tag="pj", name="pj")
                for d in range(NDC):
                    nc.tensor.matmul(
                        ps[:],
                        lhsT=wk_sb[:, d * 512 + h * HD : d * 512 + (h + 1) * HD],
                        rhs=xt[:, d * TB : (d + 1) * TB],
                        start=(d == 0),
                        stop=(d == NDC - 1),
                    )
                    if d % 2 == 1:
                        yield 430
                nc.scalar.copy(kt_sb[h][:, tb * TB : (tb + 1) * TB], ps[:])

            def v_gen(tb, tt):
                xt = x_sb[tb]
                ps = prjps.tile([128, HL * HD], f32, tag="pj", name="pj")
                for d in range(NDC):
                    nc.tensor.matmul(
                        ps[:],
                        lhsT=xt[:, d * TB + tt * 128 : d * TB + (tt + 1) * 128],
                        rhs=wv_sb[:, d * 512 : (d + 1) * 512],
                        start=(d == 0),
                        stop=(d == NDC - 1),
                    )
                    if d % 2 == 1:
                        yield 430
                nc.scalar.copy(v_sb[tb * NQT + tt][:], ps[:])

            def op_gen(qsb, tq):
                t0 = qsb * TB + tq * 128
                for dch in range(4):
                    oev = opool.tile([128, 512], bf16, tag="oev", name="oev")
                    po = prjps.tile([128, 512], f32, tag="pj", name="po")
                    for h in range(HL):
                        nc.tensor.matmul(
                            po[:],
                            lhsT=ot_sb[h][:, t0 : t0 + 128],
                            rhs=wo_sb[:, h * D + dch * 512 : h * D + (dch + 1) * 512],
                            start=(h == 0),
                            stop=(h == HL - 1),
                        )
                    if dch % 2 == 0:
                        nc.scalar.copy(oev[:], po[:])
                    else:
                        nc.vector.tensor_copy(oev[:], po[:])
                    nc.sync.dma_start(
                        out=out[t0 : t0 + 128, dch * 512 : (dch + 1) * 512], in_=oev[:]
                    )
                    yield 860

            def xdma_gen(tb):
                emit_xdma(tb)
                return
                yield  # pragma: no cover

            # filler generators per (qsb, head) slot
            F = {
                (0, 0): [v_gen(0, 0), v_gen(0, 1), v_gen(0, 2), v_gen(0, 3)],
                (0, 1): [xdma_gen(1), q_gen(1, 0), q_gen(1, 1)],
                (0, 2): [q_gen(1, 2), q_gen(1, 3), k_gen(1, 0)],
                (0, 3): [k_gen(1, 1), k_gen(1, 2), k_gen(1, 3)],
                (1, 0): [v_gen(1, 0), v_gen(1, 1), v_gen(1, 2), v_gen(1, 3)],
                (1, 1): [xdma_gen(2), q_gen(2, 0), q_gen(2, 1), op_gen(0, 0)],
                (1, 2): [q_gen(2, 2), q_gen(2, 3), k_gen(2, 0), op_gen(0, 1)],
                (1, 3): [k_gen(2, 1), k_gen(2, 2), k_gen(2, 3), op_gen(0, 2)],
                (2, 0): [v_gen(2, 0), v_gen(2, 1), v_gen(2, 2), v_gen(2, 3), op_gen(0, 3)],
                (2, 1): [xdma_gen(3), q_gen(3, 0), q_gen(3, 1), op_gen(1, 0)],
                (2, 2): [q_gen(3, 2), q_gen(3, 3), k_gen(3, 0), op_gen(1, 1)],
                (2, 3): [k_gen(3, 1), k_gen(3, 2), k_gen(3, 3), op_gen(1, 2)],
                (3, 0): [v_gen(3, 0), v_gen(3, 1), v_gen(3, 2), v_gen(3, 3), op_gen(1, 3)],
                (3, 1): [op_gen(2, 0), op_gen(2, 1)],
                (3, 2): [op_gen(2, 2)],
                (3, 3): [op_gen(2, 3)],
            }

            FILL_NS = 850

            pending = None
            for qsb in range(NTB):
                for h in range(HL):
                    work = deque()
                    work.extend(F.get((qsb, h), []))
                    if pending is not None:
                        work.append(tav_gen(*pending))
                    diffs, diags = [], []
                    sg = score_gen(h, qsb, diffs, diags)
                    for _ in sg:
                        debt = FILL_NS
                        while debt > 0 and work:
                            try:
                                debt -= next(work[0])
                            except StopIteration:
                                work.popleft()
                    while work:
                        try:
                            next(work[0])
                        except StopIteration:
                            work.popleft()
                    pending = (h, qsb, diffs, diags)
            tg = tav_gen(*pending)
            ops = deque(op_gen(3, tq) for tq in range(4))
            k = 0
            avail = 0
            for _ in tg:
                k += 1
                if k >= 14:
                    avail += 1
                for _ in range(2):
                    if avail and ops:
                        try:
                            next(ops[0])
                        except StopIteration:
                            ops.popleft()
                            avail = min(avail + 0, 3)
            while ops:
                try:
                    next(ops[0])
                except StopIteration:
                    ops.popleft()

    nc.finalize()
    return nc


def _get_nc():
    if "nc" not in _CACHE:
        _CACHE["nc"] = _build()
    return _CACHE["nc"]


def kernel(x, Wq, Wk, Wv, Wo, lambda_init):
    from concourse.bass_utils import run_bass_kernel_spmd

    bf16 = ml_dtypes.bfloat16
    x = np.asarray(x, dtype=np.float32)
    Wq = np.asarray(Wq, dtype=np.float32)
    Wk = np.asarray(Wk, dtype=np.float32)
    Wv = np.asarray(Wv, dtype=np.float32)
    Wo = np.asarray(Wo, dtype=np.float32)
    lam_full = 1.0 / (1.0 + np.exp(-np.asarray(lambda_init, dtype=np.float32)))

    msk = np.triu(np.full((128, 128), -1e30, np.float32), k=1)  # additive causal
    idnn = (-np.eye(128)).astype(bf16)  # negated identity (combine sign fix)

    # x^T rearranged per core batch: xr[p, ((tb*16)+d)*512 + j] = x[b][tb*512+j, d*128+p]
    xr_b = [
        np.ascontiguousarray(
            x[b].reshape(NTB, TB, NDC, 128).transpose(3, 0, 2, 1).reshape(128, NTB * NDC * TB)
        ).astype(bf16)
        for b in range(B)
    ]

    def wrearr(W, cols):
        # [p, d*512 + c] = W[d*128+p, cols[c]]
        Wc = W[:, cols]  # [2048, 512]
        return np.ascontiguousarray(
            Wc.reshape(NDC, 128, HL * HD).transpose(1, 0, 2).reshape(128, NDC * HL * HD)
        ).astype(bf16)

    in_maps = []
    for c in range(NCORES):
        b, g = divmod(c, NCORES // B)
        cols = slice(g * HL * HD, (g + 1) * HL * HD)
        won_r = np.ascontiguousarray(
            Wo[cols, :].reshape(HL, 128, D).transpose(1, 0, 2).reshape(128, HL * D)
        ).astype(bf16)
        in_maps.append(
            {
                "xr": xr_b[b],
                "wq": wrearr(Wq, cols),
                "wk": wrearr(Wk, cols),
                "wv": wrearr(Wv, cols),
                "won": won_r,
                "lam": np.tile(lam_full[g * HL : (g + 1) * HL], (128, 1)).astype(np.float32),
                "msk": msk,
                "idnn": idnn,
            }
        )

    nc = _get_nc()
    res = run_bass_kernel_spmd(nc, in_maps, core_ids=list(range(NCORES)))
    _CACHE["last_results"] = res

    full = np.zeros((B, T, D), np.float32)
    for c in range(NCORES):
        b = c // (NCORES // B)
        full[b] += res.results[c]["out"].astype(np.float32)
    return full

